# revision 34
# baseline (speedup 1.0000x reference)
"""ConvNearestNeightbor Trainium2 kernel.

out[b, n*C+c, i, j] = max_k |x[b,c,i-r_k,j-c_k] - neighbors[n,c,k]|
over the 9 zero-padded 3x3 shifts (r_k, c_k).

Sharding: 8 cores = 4 batch-groups x 2 num-groups.
Per core: B_loc=4 batches, N_loc=16 codebook entries.
Partition layout: (nn in 0..3, c in 0..31) -> 128 partitions, with the
codebook tile index nt in 0..3 selecting n = nt*4+nn.
Work is split into batch-halves: each (nt, half) chain runs ops of
free size 2*32*32 = 2048 (b-pair, pixels).

Per chain: 9 abs-diff planes d_k = |x_win_k - nb_k| are produced
(ScalarE Abs activation with per-partition bias -nb for most k;
VectorE tensor_scalar subtract + bitwise-and sign clear for a few),
then folded with tensor_tensor max on VectorE.
PREC="fp16" keeps d/acc in fp16 (2x DVE fold rate, one fp16 rounding
~2^-11 relative); PREC="fp32" is bit-exact vs the fp32 reference.
"""

import numpy as np

B, C, H, W = 16, 32, 32, 32
NUM = 32
NCORES = 8
BG, NG = 4, 2          # batch groups x num groups
B_LOC = B // BG        # 4
N_LOC = NUM // NG      # 16
NT = N_LOC // 4        # 4 codebook tiles of 4 n each
PH, PW = H + 2, W + 2  # 34 x 34 padded image
HB = B_LOC // 2        # batches per half-chain
FREE = HB * H * W      # 2048 per half

PREC = "fp16"          # "fp16" or "fp32"
# shifts produced on VectorE (tensor_scalar subtract + bitwise-and sign
# clear); ScalarE (Abs+bias) takes the rest. Window offsets for DVE k's
# should be 4B-aligned for fp16 4x mode: k in {0,2,3,5,6,8}.
# Alternating 3/2 per chain balances ACT vs DVE totals.
DVE_PROD_EVEN = (0, 2, 6)
DVE_PROD_ODD = (0, 2)
# k processed first, reading the unpadded xraw tile directly (the center
# window is exactly the interior) so chains start before the pad-copy.
K_ORDER = (4, 0, 1, 2, 3, 5, 6, 7, 8)

_module_cache = {}


def _build_module():
    import concourse.bacc as bacc
    import concourse.mybir as mybir
    import concourse.tile as tile

    dt = mybir.dt
    Alu = mybir.AluOpType
    AF = mybir.ActivationFunctionType

    cdt = dt.float16 if PREC == "fp16" else dt.float32
    idt = dt.uint16 if PREC == "fp16" else dt.uint32
    mask = 0x7FFF if PREC == "fp16" else 0x7FFFFFFF

    nc = bacc.Bacc("TRN2", debug=False)
    x = nc.dram_tensor("x", [B_LOC, C, H, W], dt.float32, kind="ExternalInput")
    nb = nc.dram_tensor("neighbors", [N_LOC, C, 9], dt.float32, kind="ExternalInput")
    out = nc.dram_tensor(
        "out", [B_LOC, N_LOC * C, H, W], dt.float32, kind="ExternalOutput"
    )

    # window start offsets within the padded 34x34 image for the 9 shifts
    # k = (row+1)*3 + (col+1), window starts at (1-row, 1-col)
    offs = []
    for row in (-1, 0, 1):
        for col in (-1, 0, 1):
            offs.append((1 - row, 1 - col))

    with tile.TileContext(nc) as tc:
        with (
            tc.tile_pool(name="const", bufs=1) as cpool,
            tc.tile_pool(name="accp", bufs=6) as apool,
            tc.tile_pool(name="dp", bufs=12) as dpool,
        ):
            # neighbors first: the tiny SWDGE DMA + ACT negate gate the
            # first ABS, so they must not queue behind loads/memsets
            nbt = cpool.tile([128, NT * 9], dt.float32, tag="nbt")
            # nbt[(nn,c), (t,k)] = neighbors[t*4+nn, c, k]
            nb_src = nb.ap().rearrange("(t nn) c k -> (nn c) t k", nn=4)
            nbt_v = nbt[:].rearrange("p (t k) -> p t k", t=NT)
            nc.sync.dma_start(nbt_v, nb_src)
            # raw x halves (contiguous loads; h0 split across both HWDGE
            # queues and issued before any ACT compute so the first chain
            # starts as early as possible)
            x_src = x.ap().rearrange("b c h w -> c b h w")
            xraw = []
            for h in range(2):
                t = cpool.tile([128, FREE], dt.float32, tag=f"xraw{h}")
                tv = t[:].rearrange("p (b h w) -> p b h w", b=HB, h=H, w=W)
                for nn in range(4):
                    eng = nc.scalar if nn >= 2 else nc.sync
                    eng.dma_start(
                        tv[nn * 32 : (nn + 1) * 32].rearrange("c b h w -> c b (h w)"),
                        x_src[:, h * HB : (h + 1) * HB].rearrange(
                            "c b h w -> c b (h w)"
                        ),
                    )
                xraw.append(tv)

            # negated neighbors: ACT bias computes Abs(x + (-nb))
            nbneg = cpool.tile([128, NT * 9], dt.float32, tag="nbneg")
            nc.scalar.mul(nbneg[:], nbt[:], -1.0)

            # padded halves, borders zero; interior cast-copy emitted later
            # (after the first chain's k=4 op) so ACT starts compute early
            xpad = []
            for h in range(2):
                t = cpool.tile([128, HB * PH * PW], cdt, tag=f"xpad{h}")
                nc.gpsimd.memset(t[:], 0.0)
                tv = t[:].rearrange("p (b h w) -> p b h w", b=HB, h=PH, w=PW)
                xpad.append(tv)

            def pad_copy(h):
                nc.scalar.copy(
                    xpad[h][:, :, 1 : 1 + H, 1 : 1 + W], xraw[h]
                )

            # out[b, nt*128 + p, h, w] viewed per (nt, half)
            out_v = out.ap().rearrange("b (t p) h w -> t p b (h w)", t=NT)

            chain_specs = []  # (nt, h) with h=None meaning both halves
            chain_specs += [(0, 0), (3, 0), (1, None), (2, None), (0, 1), (3, 1)]

            def chain(nt, h, dve_ks):
                """One fold chain. h=None processes both halves (4096 free)."""
                halves = [h] if h is not None else [0, 1]
                nb_free = len(halves) * HB
                acc = apool.tile([128, nb_free * H * W], cdt, tag="acc")
                nfold = 0
                first = None
                for k in K_ORDER:
                    a, bcol = offs[k]
                    d = dpool.tile([128, nb_free * H * W], cdt, tag="d")
                    d_v = d[:].rearrange("p (b h w) -> p b h w", b=nb_free, h=H, w=W)
                    for hi, hh in enumerate(halves):
                        if k == 4:
                            xwin = xraw[hh]
                        else:
                            xwin = xpad[hh][:, :, a : a + H, bcol : bcol + W]
                        dsub = d_v[:, hi * HB : (hi + 1) * HB]
                        if k in dve_ks:
                            nc.vector.tensor_scalar(
                                dsub, xwin, nbt[:, nt * 9 + k : nt * 9 + k + 1],
                                None, Alu.subtract,
                            )
                        else:
                            nc.scalar.activation(
                                dsub, xwin, AF.Abs,
                                bias=nbneg[:, nt * 9 + k : nt * 9 + k + 1],
                                scale=1.0,
                            )
                    if k in dve_ks:
                        nc.vector.tensor_scalar(
                            d[:].bitcast(idt), d[:].bitcast(idt), mask, None,
                            Alu.bitwise_and,
                        )
                    if first is None:
                        first = d
                    elif nfold == 0:
                        nc.vector.tensor_tensor(acc[:], first[:], d[:], Alu.max)
                        nfold = 1
                    else:
                        nc.vector.tensor_tensor(acc[:], acc[:], d[:], Alu.max)
                acc_s = acc[:].rearrange("p (b s) -> p b s", b=nb_free)
                if h is not None:
                    dst = out_v[nt][:, h * HB : (h + 1) * HB]
                else:
                    dst = out_v[nt]
                if PREC == "fp16":
                    nc.gpsimd.dma_start(dst, acc_s)  # SWDGE cast fp16->f32
                else:
                    nc.sync.dma_start(dst, acc_s)

            pad_copy(0)
            for ci, (nt, h) in enumerate(chain_specs):
                dve_ks = DVE_PROD_EVEN if ci % 2 == 0 else DVE_PROD_ODD
                chain(nt, h, dve_ks)
                if ci == 0:
                    pad_copy(1)

    nc.compile()
    return nc


def _get_module():
    if "nc" not in _module_cache:
        _module_cache["nc"] = _build_module()
    return _module_cache["nc"]


def _run(x, neighbors, trace=False):
    from concourse import bass_utils

    x = np.ascontiguousarray(x, dtype=np.float32)
    neighbors = np.ascontiguousarray(neighbors, dtype=np.float32)
    in_maps = []
    for core in range(NCORES):
        bg, ng = divmod(core, NG)
        in_maps.append(
            {
                "x": x[bg * B_LOC : (bg + 1) * B_LOC],
                "neighbors": neighbors[ng * N_LOC : (ng + 1) * N_LOC],
            }
        )
    res = bass_utils.run_bass_kernel_spmd(
        _get_module(), in_maps, core_ids=list(range(NCORES)), trace=trace
    )
    out = np.empty((B, NUM * C, H, W), dtype=np.float32)
    for core in range(NCORES):
        bg, ng = divmod(core, NG)
        out[bg * B_LOC : (bg + 1) * B_LOC, ng * N_LOC * C : (ng + 1) * N_LOC * C] = (
            res.results[core]["out"]
        )
    return out, res


def kernel(x, neighbors):
    out, _ = _run(x, neighbors, trace=False)
    return out


# revision 35
# speedup vs baseline: 1.0016x; 1.0016x over previous
"""ConvNearestNeightbor Trainium2 kernel.

out[b, n*C+c, i, j] = max_k |x[b,c,i-r_k,j-c_k] - neighbors[n,c,k]|
over the 9 zero-padded 3x3 shifts (r_k, c_k).

Sharding: 8 cores = 4 batch-groups x 2 num-groups.
Per core: B_loc=4 batches, N_loc=16 codebook entries.
Partition layout: (nn in 0..3, c in 0..31) -> 128 partitions, with the
codebook tile index nt in 0..3 selecting n = nt*4+nn.
Work is split into batch-halves: each (nt, half) chain runs ops of
free size 2*32*32 = 2048 (b-pair, pixels).

Per chain: 9 abs-diff planes d_k = |x_win_k - nb_k| are produced
(ScalarE Abs activation with per-partition bias -nb for most k;
VectorE tensor_scalar subtract + bitwise-and sign clear for a few),
then folded with tensor_tensor max on VectorE.
PREC="fp16" keeps d/acc in fp16 (2x DVE fold rate, one fp16 rounding
~2^-11 relative); PREC="fp32" is bit-exact vs the fp32 reference.
"""

import numpy as np

B, C, H, W = 16, 32, 32, 32
NUM = 32
NCORES = 8
BG, NG = 4, 2          # batch groups x num groups
B_LOC = B // BG        # 4
N_LOC = NUM // NG      # 16
NT = N_LOC // 4        # 4 codebook tiles of 4 n each
PH, PW = H + 2, W + 2  # 34 x 34 padded image
HB = B_LOC // 2        # batches per half-chain
FREE = HB * H * W      # 2048 per half

PREC = "fp16"          # "fp16" or "fp32"
# shifts produced on VectorE (tensor_scalar subtract + bitwise-and sign
# clear); ScalarE (Abs+bias) takes the rest. Window offsets for DVE k's
# should be 4B-aligned for fp16 4x mode: k in {0,2,3,5,6,8}.
# Alternating 3/2 per chain balances ACT vs DVE totals.
DVE_PROD_EVEN = (0, 2, 6)
DVE_PROD_ODD = (0, 2)
# k processed first, reading the unpadded xraw tile directly (the center
# window is exactly the interior) so chains start before the pad-copy.
K_ORDER = (4, 0, 1, 2, 3, 5, 6, 7, 8)

_module_cache = {}


def _build_module():
    import concourse.bacc as bacc
    import concourse.mybir as mybir
    import concourse.tile as tile

    dt = mybir.dt
    Alu = mybir.AluOpType
    AF = mybir.ActivationFunctionType

    cdt = dt.float16 if PREC == "fp16" else dt.float32
    idt = dt.uint16 if PREC == "fp16" else dt.uint32
    mask = 0x7FFF if PREC == "fp16" else 0x7FFFFFFF

    nc = bacc.Bacc("TRN2", debug=False)
    x = nc.dram_tensor("x", [B_LOC, C, H, W], dt.float32, kind="ExternalInput")
    nb = nc.dram_tensor("neighbors", [N_LOC, C, 9], dt.float32, kind="ExternalInput")
    out = nc.dram_tensor(
        "out", [B_LOC, N_LOC * C, H, W], dt.float32, kind="ExternalOutput"
    )

    # window start offsets within the padded 34x34 image for the 9 shifts
    # k = (row+1)*3 + (col+1), window starts at (1-row, 1-col)
    offs = []
    for row in (-1, 0, 1):
        for col in (-1, 0, 1):
            offs.append((1 - row, 1 - col))

    with tile.TileContext(nc) as tc:
        with (
            tc.tile_pool(name="const", bufs=1) as cpool,
            tc.tile_pool(name="accp", bufs=5) as apool,
            tc.tile_pool(name="dp", bufs=10) as dpool,
        ):
            # neighbors first: the tiny SWDGE DMA + ACT negate gate the
            # first ABS, so they must not queue behind loads/memsets
            nbt = cpool.tile([128, NT * 9], dt.float32, tag="nbt")
            # nbt[(nn,c), (t,k)] = neighbors[t*4+nn, c, k]
            nb_src = nb.ap().rearrange("(t nn) c k -> (nn c) t k", nn=4)
            nbt_v = nbt[:].rearrange("p (t k) -> p t k", t=NT)
            nc.sync.dma_start(nbt_v, nb_src)
            # raw x halves (contiguous loads; h0 split across both HWDGE
            # queues and issued before any ACT compute so the first chain
            # starts as early as possible)
            x_src = x.ap().rearrange("b c h w -> c b h w")
            xraw = []
            for h in range(2):
                t = cpool.tile([128, FREE], dt.float32, tag=f"xraw{h}")
                tv = t[:].rearrange("p (b h w) -> p b h w", b=HB, h=H, w=W)
                for nn in range(4):
                    eng = nc.scalar if nn >= 2 else nc.sync
                    eng.dma_start(
                        tv[nn * 32 : (nn + 1) * 32].rearrange("c b h w -> c b (h w)"),
                        x_src[:, h * HB : (h + 1) * HB].rearrange(
                            "c b h w -> c b (h w)"
                        ),
                    )
                xraw.append(tv)

            # negated neighbors: ACT bias computes Abs(x + (-nb))
            nbneg = cpool.tile([128, NT * 9], dt.float32, tag="nbneg")
            nc.scalar.mul(nbneg[:], nbt[:], -1.0)

            # padded halves, borders zero; interior cast-copy emitted later
            # (after the first chain's k=4 op) so ACT starts compute early
            xpad = []
            for h in range(2):
                t = cpool.tile([128, HB * PH * PW], cdt, tag=f"xpad{h}")
                nc.gpsimd.memset(t[:], 0.0)
                tv = t[:].rearrange("p (b h w) -> p b h w", b=HB, h=PH, w=PW)
                xpad.append(tv)

            def pad_copy(h):
                nc.scalar.copy(
                    xpad[h][:, :, 1 : 1 + H, 1 : 1 + W], xraw[h]
                )

            # out[b, nt*128 + p, h, w] viewed per (nt, half)
            out_v = out.ap().rearrange("b (t p) h w -> t p b (h w)", t=NT)

            chain_specs = []  # (nt, h) with h=None meaning both halves
            chain_specs += [(0, 0), (3, 0), (1, None), (2, None), (0, 1), (3, 1)]

            def chain(nt, h, dve_ks):
                """One fold chain. h=None processes both halves (4096 free)."""
                halves = [h] if h is not None else [0, 1]
                nb_free = len(halves) * HB
                acc = apool.tile([128, nb_free * H * W], cdt, tag="acc")
                nfold = 0
                first = None
                for k in K_ORDER:
                    a, bcol = offs[k]
                    d = dpool.tile([128, nb_free * H * W], cdt, tag="d")
                    d_v = d[:].rearrange("p (b h w) -> p b h w", b=nb_free, h=H, w=W)
                    for hi, hh in enumerate(halves):
                        if k == 4:
                            xwin = xraw[hh]
                        else:
                            xwin = xpad[hh][:, :, a : a + H, bcol : bcol + W]
                        dsub = d_v[:, hi * HB : (hi + 1) * HB]
                        if k in dve_ks:
                            nc.vector.tensor_scalar(
                                dsub, xwin, nbt[:, nt * 9 + k : nt * 9 + k + 1],
                                None, Alu.subtract,
                            )
                        else:
                            nc.scalar.activation(
                                dsub, xwin, AF.Abs,
                                bias=nbneg[:, nt * 9 + k : nt * 9 + k + 1],
                                scale=1.0,
                            )
                    if k in dve_ks:
                        nc.vector.tensor_scalar(
                            d[:].bitcast(idt), d[:].bitcast(idt), mask, None,
                            Alu.bitwise_and,
                        )
                    if first is None:
                        first = d
                    elif nfold == 0:
                        nc.vector.tensor_tensor(acc[:], first[:], d[:], Alu.max)
                        nfold = 1
                    else:
                        nc.vector.tensor_tensor(acc[:], acc[:], d[:], Alu.max)
                acc_s = acc[:].rearrange("p (b s) -> p b s", b=nb_free)
                if h is not None:
                    dst = out_v[nt][:, h * HB : (h + 1) * HB]
                else:
                    dst = out_v[nt]
                if PREC == "fp16":
                    nc.gpsimd.dma_start(dst, acc_s)  # SWDGE cast fp16->f32
                else:
                    nc.sync.dma_start(dst, acc_s)

            pad_copy(0)
            for ci, (nt, h) in enumerate(chain_specs):
                dve_ks = DVE_PROD_EVEN if ci % 2 == 0 else DVE_PROD_ODD
                chain(nt, h, dve_ks)
                if ci == 0:
                    pad_copy(1)

    nc.compile()
    return nc


def _get_module():
    if "nc" not in _module_cache:
        _module_cache["nc"] = _build_module()
    return _module_cache["nc"]


def _run(x, neighbors, trace=False):
    from concourse import bass_utils

    x = np.ascontiguousarray(x, dtype=np.float32)
    neighbors = np.ascontiguousarray(neighbors, dtype=np.float32)
    in_maps = []
    for core in range(NCORES):
        bg, ng = divmod(core, NG)
        in_maps.append(
            {
                "x": x[bg * B_LOC : (bg + 1) * B_LOC],
                "neighbors": neighbors[ng * N_LOC : (ng + 1) * N_LOC],
            }
        )
    res = bass_utils.run_bass_kernel_spmd(
        _get_module(), in_maps, core_ids=list(range(NCORES)), trace=trace
    )
    out = np.empty((B, NUM * C, H, W), dtype=np.float32)
    for core in range(NCORES):
        bg, ng = divmod(core, NG)
        out[bg * B_LOC : (bg + 1) * B_LOC, ng * N_LOC * C : (ng + 1) * N_LOC * C] = (
            res.results[core]["out"]
        )
    return out, res


def kernel(x, neighbors):
    out, _ = _run(x, neighbors, trace=False)
    return out
